# revision 19
# baseline (speedup 1.0000x reference)
"""CLIP contrastive loss on 8 Trainium2 NeuronCores (Bass/Tile), fp8 DoubleRow.

Strategy (data-parallel over image rows, hint's local_loss path):
  - Core c holds image rows [c*1024, (c+1)*1024) and the FULL text matrix.
  - Text rows are rolled by c*1024 on the host so every core's diagonal
    block sits at local cols [0, 1024) (the compiled program is
    core-independent).
  - Features are scaled by 16 on the host and quantized to fp8 e4m3; the
    PE runs DoubleRow matmuls (2 fp8 k-planes per cell, K=256 per MM) at
    ~2x bf16 throughput.  PSUM accumulates exact f32; logits = psum *
    (scale/256).
  - Loop: for each 2048-col group (4 of them), for each 128-row m-tile
    (8): 2 kc x (2048/MM_W) DoubleRow MMs -> [128, 2048] PSUM (4 banks),
    then ONE ACT exp over the whole span (bf16 out, accum_out = partial
    row sums), and a DVE add into the group's column accumulator.
  - Diagonals all live in group 0 (cols mt*128..mt*128+128): DVE
    tensor_mul with (scale/256)*I + free-axis reduce.
  - Host: partition-reduce column accumulators, combine row/col exp-sums
    and diagonals in float64: lse = shift + log(sum); mean over both
    directions.

Fixed-shift logsumexp is numerically safe: logits are bounded by +-scale
and shift = scale/2 keeps every term that matters in normal f32 range.
"""

from contextlib import ExitStack

import numpy as np
import ml_dtypes

import concourse.bass as bass
from concourse import bacc
import concourse.tile as tile
from concourse import mybir
from concourse.bass import ts
from concourse.bass_utils import run_bass_kernel_spmd

N = 8192
D = 512
NC = 8
M_LOC = N // NC          # 1024 image rows per core
MT = M_LOC // 128        # 8 m-tiles of 128 rows
NG = 4                   # column groups
GW = N // NG             # 2048 cols per group
KC = 2                   # DoubleRow K-chunks (256 each)
PRE = 16.0               # host-side fp8 pre-scale per operand

F32 = mybir.dt.float32
BF16 = mybir.dt.bfloat16
FP8 = mybir.dt.float8e4

# moving-free per matmul: out width = MM_W, moving fp8 elements = 2*MM_W
MM_W = 512

_CACHE = {}
LAST_RESULTS = None


def _build(scale: float, shift: float, mm_w: int):
    n_mm = GW // mm_w            # matmuls per (kc, group-span)
    act_scale = scale / (PRE * PRE)
    nc = bacc.Bacc("TRN2", debug=False)

    at_d = nc.dram_tensor("at_in", [128, KC, 2, M_LOC], FP8, kind="ExternalInput").ap()
    bt_d = nc.dram_tensor("bt_in", [NG, KC, 128, 2, GW], FP8, kind="ExternalInput").ap()
    eye_d = nc.dram_tensor("eye_in", [128, 128], F32, kind="ExternalInput").ap()

    rowr_d = nc.dram_tensor("rowr_out", [128, MT, NG], F32, kind="ExternalOutput").ap()
    colsum_d = nc.dram_tensor("colsum_out", [NG, 128, GW], BF16, kind="ExternalOutput").ap()
    diag_d = nc.dram_tensor("diag_out", [128, MT], F32, kind="ExternalOutput").ap()

    with ExitStack() as ctx:
        tc = ctx.enter_context(tile.TileContext(nc))
        singles = ctx.enter_context(tc.tile_pool(name="singles", bufs=1))
        btp = ctx.enter_context(tc.tile_pool(name="btp", bufs=NG * KC))
        expp = ctx.enter_context(tc.tile_pool(name="expp", bufs=4))
        cap = ctx.enter_context(tc.tile_pool(name="cap", bufs=2))
        scr = ctx.enter_context(tc.tile_pool(name="scr", bufs=2))
        psum = ctx.enter_context(tc.tile_pool(name="psum", bufs=2, space="PSUM"))

        at_t = singles.tile([128, KC, 2, M_LOC], FP8)
        bt_tiles = [
            [btp.tile([128, 2, GW], FP8, name=f"bt{g}_{kc}", tag="bt") for kc in range(KC)]
            for g in range(NG)
        ]
        # Load order tuned for fastest first-matmul: the first MM needs only
        # at[mt0, kc0] (32KB) and bt00's first 512 cols (128KB); stream the
        # rest in consumption order.
        nc.sync.dma_start(at_t[:, 0, :, 0:128], at_d[:, 0, :, 0:128])
        nc.sync.dma_start(bt_tiles[0][0][:, :, 0:512], bt_d[0, 0][:, :, 0:512])
        nc.sync.dma_start(at_t[:, 0, :, 128:M_LOC], at_d[:, 0, :, 128:M_LOC])
        nc.sync.dma_start(bt_tiles[0][0][:, :, 512:1024], bt_d[0, 0][:, :, 512:1024])
        nc.sync.dma_start(bt_tiles[0][0][:, :, 1024:GW], bt_d[0, 0][:, :, 1024:GW])
        nc.sync.dma_start(at_t[:, 1], at_d[:, 1])
        for h in range(2):
            nc.sync.dma_start(
                bt_tiles[0][1][:, :, ts(h, GW // 2)], bt_d[0, 1][:, :, ts(h, GW // 2)]
            )
        eye_t = singles.tile([128, 128], F32)
        nc.sync.dma_start(eye_t, eye_d)
        bias_t = singles.tile([128, 1], F32)
        nc.vector.memset(bias_t, -shift)
        rowr = singles.tile([128, MT, NG], F32)
        diag_sb = singles.tile([128, MT], F32)

        for g in range(1, NG):
            for kc in range(KC):
                nc.sync.dma_start(bt_tiles[g][kc], bt_d[g, kc])

        for g in range(NG):
            colacc = cap.tile([128, GW], BF16, name=f"cacc{g}", tag="cacc")
            for mt in range(MT):
                s_ps = psum.tile([128, GW], F32, name=f"s{g}_{mt}", tag="spsum")
                for kc in range(KC):
                    lhsT = at_t[:, kc, :, ts(mt, 128)]          # [128, 2, 128]
                    for w in range(n_mm):
                        nc.tensor.matmul(
                            s_ps[:, ts(w, mm_w)],
                            lhsT,
                            bt_tiles[g][kc][:, :, ts(w, mm_w)],  # [128, 2, mm_w]
                            start=(kc == 0),
                            stop=(kc == KC - 1),
                            perf_mode=mybir.MatmulPerfMode.DoubleRow,
                        )
                if g == 0:
                    # diag block for mt sits at local cols [mt*128, mt*128+128)
                    dscr = scr.tile([128, 128], F32, name=f"dscr{mt}", tag="dscr")
                    nc.vector.tensor_mul(dscr, s_ps[:, ts(mt, 128)], eye_t)
                    nc.vector.tensor_reduce(
                        out=diag_sb[:, mt : mt + 1],
                        in_=dscr,
                        axis=mybir.AxisListType.X,
                        op=mybir.AluOpType.add,
                    )
                e_t = expp.tile([128, GW], BF16, name=f"e{g}_{mt}", tag="exp")
                # Row sums: ACT accum for half the groups, DVE reduce for the
                # other half, to balance the two engines (ACT is the
                # bottleneck; its READ_ACCUMULATOR costs ~283ns/slot).
                rowsum_on_act = g < NG // 2
                nc.scalar.activation(
                    e_t,
                    s_ps,
                    mybir.ActivationFunctionType.Exp,
                    bias=bias_t,
                    scale=act_scale,
                    accum_out=rowr[:, mt, g : g + 1] if rowsum_on_act else None,
                )
                if mt == 0:
                    nc.vector.tensor_copy(colacc, e_t)
                else:
                    nc.vector.tensor_add(colacc, colacc, e_t)
                if not rowsum_on_act:
                    nc.vector.tensor_reduce(
                        out=rowr[:, mt, g : g + 1],
                        in_=e_t,
                        axis=mybir.AxisListType.X,
                        op=mybir.AluOpType.add,
                    )
            nc.sync.dma_start(colsum_d[g], colacc)

        nc.sync.dma_start(rowr_d, rowr)
        nc.sync.dma_start(diag_d, diag_sb)

    nc.compile()
    return nc


def _prep_inputs(img, txt, scale):
    fp8 = ml_dtypes.float8_e4m3fn
    eye = ((scale / (PRE * PRE)) * np.eye(128)).astype(np.float32)
    in_maps = []
    for c in range(NC):
        A = (PRE * img[c * M_LOC : (c + 1) * M_LOC]).astype(fp8)   # [1024, 512]
        # k = kc*256 + ko*128 + p
        at = np.ascontiguousarray(
            A.T.reshape(KC, 2, 128, M_LOC).transpose(2, 0, 1, 3)
        )                                                          # [128, KC, 2, 1024]
        tr = np.roll(txt, -c * M_LOC, axis=0)                      # local col j -> global (j + c*1024) % N
        B = (PRE * tr).astype(fp8)                                 # [8192, 512]
        bt = np.ascontiguousarray(
            B.T.reshape(KC, 2, 128, NG, GW).transpose(3, 0, 2, 1, 4)
        )                                                          # [NG, KC, 128, 2, GW]
        in_maps.append({"at_in": at, "bt_in": bt, "eye_in": eye})
    return in_maps


def kernel(image_features, text_features, logit_scale):
    global LAST_RESULTS
    img = np.ascontiguousarray(np.asarray(image_features, dtype=np.float32))
    txt = np.ascontiguousarray(np.asarray(text_features, dtype=np.float32))
    scale = float(np.asarray(logit_scale))
    shift = 0.5 * scale

    key = (scale, MM_W)
    if key not in _CACHE:
        _CACHE[key] = _build(scale, shift, MM_W)
    nc = _CACHE[key]

    in_maps = _prep_inputs(img, txt, scale)
    res = run_bass_kernel_spmd(nc, in_maps, core_ids=list(range(NC)))
    LAST_RESULTS = res

    colsum_tot = np.zeros(N, dtype=np.float64)
    lse_rows = []
    diags = []
    for c, r in enumerate(res.results):
        rowsum = r["rowr_out"].astype(np.float64).sum(axis=2)       # [128, MT]
        lse_rows.append(shift + np.log(rowsum.T.reshape(-1)))       # row = mt*128 + p
        diags.append(r["diag_out"].astype(np.float64).T.reshape(-1))
        colsum_tot += np.roll(
            r["colsum_out"].astype(np.float64).sum(axis=1).reshape(-1), c * M_LOC
        )
    lse_row = np.concatenate(lse_rows)
    diag = np.concatenate(diags)
    lse_col = shift + np.log(colsum_tot)

    loss = 0.5 * (np.mean(lse_row - diag) + np.mean(lse_col - diag))
    return np.float32(loss)


# revision 24
# speedup vs baseline: 1.0452x; 1.0452x over previous
"""CLIP contrastive loss on 8 Trainium2 NeuronCores (Bass/Tile), fp8 DoubleRow.

Strategy (data-parallel over image rows, hint's local_loss path):
  - Core c holds image rows [c*1024, (c+1)*1024) and the FULL text matrix.
  - Text rows are rolled by c*1024 on the host so every core's diagonal
    block sits at local cols [0, 1024) (the compiled program is
    core-independent).
  - Features are scaled by 16 on the host and quantized to fp8 e4m3; the
    PE runs DoubleRow matmuls (2 fp8 k-planes per cell, K=256 per MM) at
    ~2x bf16 throughput.  PSUM accumulates exact f32; logits = psum *
    (scale/256).
  - Loop: for each 2048-col group (4 of them), for each 128-row m-tile
    (8): 2 kc x (2048/MM_W) DoubleRow MMs -> [128, 2048] PSUM (4 banks),
    then ONE ACT exp over the whole span (bf16 out, accum_out = partial
    row sums), and a DVE add into the group's column accumulator.
  - Diagonals all live in group 0 (cols mt*128..mt*128+128): DVE
    tensor_mul with (scale/256)*I + free-axis reduce.
  - Host: partition-reduce column accumulators, combine row/col exp-sums
    and diagonals in float64: lse = shift + log(sum); mean over both
    directions.

Fixed-shift logsumexp is numerically safe: logits are bounded by +-scale
and shift = scale/2 keeps every term that matters in normal f32 range.
"""

from contextlib import ExitStack

import numpy as np
import ml_dtypes

import concourse.bass as bass
from concourse import bacc
import concourse.tile as tile
from concourse import mybir
from concourse.bass import ts
from concourse.bass_utils import run_bass_kernel_spmd

N = 8192
D = 512
NC = 8
M_LOC = N // NC          # 1024 image rows per core
MT = M_LOC // 128        # 8 m-tiles of 128 rows
NG = 4                   # column groups
GW = N // NG             # 2048 cols per group
KC = 2                   # DoubleRow K-chunks (256 each)
PRE = 16.0               # host-side fp8 pre-scale per operand

F32 = mybir.dt.float32
BF16 = mybir.dt.bfloat16
FP8 = mybir.dt.float8e4

# moving-free per matmul: out width = MM_W, moving fp8 elements = 2*MM_W
MM_W = 512

_CACHE = {}
LAST_RESULTS = None


def _build(scale: float, shift: float, mm_w: int):
    n_mm = GW // mm_w            # matmuls per (kc, group-span)
    act_scale = scale / (PRE * PRE)
    nc = bacc.Bacc("TRN2", debug=False)

    at_d = nc.dram_tensor("at_in", [128, KC, 2, M_LOC], FP8, kind="ExternalInput").ap()
    bt_d = nc.dram_tensor("bt_in", [NG, KC, 128, 2, GW], FP8, kind="ExternalInput").ap()
    eye_d = nc.dram_tensor("eye_in", [128, 128], F32, kind="ExternalInput").ap()

    rowr_d = nc.dram_tensor("rowr_out", [128, MT, NG], F32, kind="ExternalOutput").ap()
    colsum_d = nc.dram_tensor("colsum_out", [NG, 128, GW], BF16, kind="ExternalOutput").ap()
    diag_d = nc.dram_tensor("diag_out", [128, MT], F32, kind="ExternalOutput").ap()

    with ExitStack() as ctx:
        tc = ctx.enter_context(tile.TileContext(nc))
        singles = ctx.enter_context(tc.tile_pool(name="singles", bufs=1))
        btp = ctx.enter_context(tc.tile_pool(name="btp", bufs=NG * KC))
        expp = ctx.enter_context(tc.tile_pool(name="expp", bufs=4))
        cap = ctx.enter_context(tc.tile_pool(name="cap", bufs=2))
        scr = ctx.enter_context(tc.tile_pool(name="scr", bufs=2))
        psum = ctx.enter_context(tc.tile_pool(name="psum", bufs=2, space="PSUM"))

        at_t = singles.tile([128, KC, 2, M_LOC], FP8)
        bt_tiles = [
            [btp.tile([128, 2, GW], FP8, name=f"bt{g}_{kc}", tag="bt") for kc in range(KC)]
            for g in range(NG)
        ]
        # Load order tuned for fastest first-matmul: the first MM needs only
        # at[mt0, kc0] (32KB) and bt00's first 512 cols (128KB); stream the
        # rest in consumption order.
        nc.sync.dma_start(at_t[:, 0, :, 0:128], at_d[:, 0, :, 0:128])
        nc.sync.dma_start(bt_tiles[0][0][:, :, 0:512], bt_d[0, 0][:, :, 0:512])
        nc.sync.dma_start(at_t[:, 0, :, 128:M_LOC], at_d[:, 0, :, 128:M_LOC])
        nc.sync.dma_start(bt_tiles[0][0][:, :, 512:1024], bt_d[0, 0][:, :, 512:1024])
        nc.sync.dma_start(bt_tiles[0][0][:, :, 1024:GW], bt_d[0, 0][:, :, 1024:GW])
        nc.sync.dma_start(at_t[:, 1], at_d[:, 1])
        for h in range(2):
            nc.sync.dma_start(
                bt_tiles[0][1][:, :, ts(h, GW // 2)], bt_d[0, 1][:, :, ts(h, GW // 2)]
            )
        eye_t = singles.tile([128, 128], F32)
        nc.sync.dma_start(eye_t, eye_d)
        bias_t = singles.tile([128, 1], F32)
        nc.vector.memset(bias_t, -shift)
        zeros_t = singles.tile([128, GW], BF16)
        nc.vector.memset(zeros_t, 0.0)
        rowr = singles.tile([128, MT, NG], F32)
        diag_sb = singles.tile([128, MT], F32)

        for g in range(1, NG):
            for kc in range(KC):
                nc.sync.dma_start(bt_tiles[g][kc], bt_d[g, kc])

        for g in range(NG):
            # STT's accum_out is wrong when out aliases in1, so the column
            # accumulator ping-pongs between two buffers.
            cacc = [
                cap.tile([128, GW], BF16, name=f"cacc{g}_{i}", tag=f"cacc{i}")
                for i in range(2)
            ]
            for mt in range(MT):
                colacc = cacc[mt % 2]
                prev = zeros_t if mt == 0 else cacc[(mt + 1) % 2]
                s_ps = psum.tile([128, GW], F32, name=f"s{g}_{mt}", tag="spsum")
                for kc in range(KC):
                    lhsT = at_t[:, kc, :, ts(mt, 128)]          # [128, 2, 128]
                    for w in range(n_mm):
                        nc.tensor.matmul(
                            s_ps[:, ts(w, mm_w)],
                            lhsT,
                            bt_tiles[g][kc][:, :, ts(w, mm_w)],  # [128, 2, mm_w]
                            start=(kc == 0),
                            stop=(kc == KC - 1),
                            perf_mode=mybir.MatmulPerfMode.DoubleRow,
                        )
                if g == 0:
                    # diag block for mt sits at local cols [mt*128, mt*128+128)
                    dscr = scr.tile([128, 128], F32, name=f"dscr{mt}", tag="dscr")
                    nc.vector.tensor_mul(dscr, s_ps[:, ts(mt, 128)], eye_t)
                    nc.vector.tensor_reduce(
                        out=diag_sb[:, mt : mt + 1],
                        in_=dscr,
                        axis=mybir.AxisListType.X,
                        op=mybir.AluOpType.add,
                    )
                e_t = expp.tile([128, GW], BF16, name=f"e{g}_{mt}", tag="exp")
                nc.scalar.activation(
                    e_t,
                    s_ps,
                    mybir.ActivationFunctionType.Exp,
                    bias=bias_t,
                    scale=act_scale,
                )
                # Fused colacc += e with running row-total (accum_out);
                # host recovers per-slot row sums by differencing the
                # consecutive totals.  Keeps ACT free of READ_ACCUMULATOR.
                nc.vector.scalar_tensor_tensor(
                    out=colacc,
                    in0=e_t,
                    scalar=1.0,
                    in1=prev,
                    op0=mybir.AluOpType.mult,
                    op1=mybir.AluOpType.add,
                    accum_out=rowr[:, mt, g : g + 1],
                )
            nc.sync.dma_start(colsum_d[g], cacc[(MT - 1) % 2])

        nc.sync.dma_start(rowr_d, rowr)
        nc.sync.dma_start(diag_d, diag_sb)

    nc.compile()
    return nc


def _prep_inputs(img, txt, scale):
    fp8 = ml_dtypes.float8_e4m3fn
    eye = ((scale / (PRE * PRE)) * np.eye(128)).astype(np.float32)
    in_maps = []
    for c in range(NC):
        A = (PRE * img[c * M_LOC : (c + 1) * M_LOC]).astype(fp8)   # [1024, 512]
        # k = kc*256 + ko*128 + p
        at = np.ascontiguousarray(
            A.T.reshape(KC, 2, 128, M_LOC).transpose(2, 0, 1, 3)
        )                                                          # [128, KC, 2, 1024]
        tr = np.roll(txt, -c * M_LOC, axis=0)                      # local col j -> global (j + c*1024) % N
        B = (PRE * tr).astype(fp8)                                 # [8192, 512]
        bt = np.ascontiguousarray(
            B.T.reshape(KC, 2, 128, NG, GW).transpose(3, 0, 2, 1, 4)
        )                                                          # [NG, KC, 128, 2, GW]
        in_maps.append({"at_in": at, "bt_in": bt, "eye_in": eye})
    return in_maps


def kernel(image_features, text_features, logit_scale):
    global LAST_RESULTS
    img = np.ascontiguousarray(np.asarray(image_features, dtype=np.float32))
    txt = np.ascontiguousarray(np.asarray(text_features, dtype=np.float32))
    scale = float(np.asarray(logit_scale))
    shift = 0.5 * scale

    key = (scale, MM_W)
    if key not in _CACHE:
        _CACHE[key] = _build(scale, shift, MM_W)
    nc = _CACHE[key]

    in_maps = _prep_inputs(img, txt, scale)
    res = run_bass_kernel_spmd(nc, in_maps, core_ids=list(range(NC)))
    LAST_RESULTS = res

    colsum_tot = np.zeros(N, dtype=np.float64)
    lse_rows = []
    diags = []
    for c, r in enumerate(res.results):
        rowr = r["rowr_out"].astype(np.float64)                     # [128, MT, NG] running totals
        rowsum = np.diff(rowr, axis=1, prepend=0.0).sum(axis=2)     # [128, MT]
        lse_rows.append(shift + np.log(rowsum.T.reshape(-1)))       # row = mt*128 + p
        diags.append(r["diag_out"].astype(np.float64).T.reshape(-1))
        colsum_tot += np.roll(
            r["colsum_out"].astype(np.float64).sum(axis=1).reshape(-1), c * M_LOC
        )
    lse_row = np.concatenate(lse_rows)
    diag = np.concatenate(diags)
    lse_col = shift + np.log(colsum_tot)

    loss = 0.5 * (np.mean(lse_row - diag) + np.mean(lse_col - diag))
    return np.float32(loss)


# revision 30
# speedup vs baseline: 1.1768x; 1.1260x over previous
"""CLIP contrastive loss on 8 Trainium2 NeuronCores (Bass/Tile), fp8 DoubleRow.

Strategy (data-parallel over image rows, hint's local_loss path):
  - Core c holds image rows [c*1024, (c+1)*1024) and the FULL text matrix.
  - Text rows are rolled by c*1024 on the host so every core's diagonal
    block sits at local cols [0, 1024) (the compiled program is
    core-independent).
  - Features are scaled by 16 on the host and quantized to fp8 e4m3; the
    PE runs DoubleRow matmuls (2 fp8 k-planes per cell, K=256 per MM) at
    ~2x bf16 throughput.  PSUM accumulates exact f32; logits = psum *
    (scale/256).
  - Loop: for each 2048-col group (4 of them), for each 128-row m-tile
    (8): 2 kc x (2048/MM_W) DoubleRow MMs -> [128, 2048] PSUM (4 banks),
    then ONE ACT exp over the whole span (bf16 out, accum_out = partial
    row sums), and a DVE add into the group's column accumulator.
  - Diagonals all live in group 0 (cols mt*128..mt*128+128): DVE
    tensor_mul with (scale/256)*I + free-axis reduce.
  - Host: partition-reduce column accumulators, combine row/col exp-sums
    and diagonals in float64: lse = shift + log(sum); mean over both
    directions.

Fixed-shift logsumexp is numerically safe: logits are bounded by +-scale
and shift = scale/2 keeps every term that matters in normal f32 range.
"""

from contextlib import ExitStack

import numpy as np
import ml_dtypes

import concourse.bass as bass
from concourse import bacc
import concourse.tile as tile
from concourse import mybir
from concourse.bass import ts
from concourse.bass_utils import run_bass_kernel_spmd

N = 8192
D = 512
NC = 8
M_LOC = N // NC          # 1024 image rows per core
MT = M_LOC // 128        # 8 m-tiles of 128 rows
NG = 4                   # column groups
GW = N // NG             # 2048 cols per group
KC = 2                   # DoubleRow K-chunks (256 each)
PRE = 16.0               # host-side fp8 pre-scale per operand

F32 = mybir.dt.float32
BF16 = mybir.dt.bfloat16
FP8 = mybir.dt.float8e4

# moving-free per matmul: out width = MM_W, moving fp8 elements = 2*MM_W
MM_W = 512

_CACHE = {}
LAST_RESULTS = None


def _build(scale: float, shift: float, mm_w: int):
    n_mm = GW // mm_w            # matmuls per (kc, group-span)
    act_scale = scale / (PRE * PRE)
    nc = bacc.Bacc("TRN2", debug=False)

    at_d = nc.dram_tensor("at_in", [128, KC, 2, M_LOC], FP8, kind="ExternalInput").ap()
    bt_d = nc.dram_tensor("bt_in", [NG, KC, 128, 2, GW], FP8, kind="ExternalInput").ap()
    eye_d = nc.dram_tensor("eye_in", [128, 128], F32, kind="ExternalInput").ap()

    rowr_d = nc.dram_tensor("rowr_out", [128, MT, NG], F32, kind="ExternalOutput").ap()
    rowl_d = nc.dram_tensor("rowl_out", [128, 2], F32, kind="ExternalOutput").ap()
    colsum_d = nc.dram_tensor("colsum_out", [NG, 128, GW], BF16, kind="ExternalOutput").ap()
    diag_d = nc.dram_tensor("diag_out", [128, MT], F32, kind="ExternalOutput").ap()

    with ExitStack() as ctx:
        tc = ctx.enter_context(tile.TileContext(nc))
        singles = ctx.enter_context(tc.tile_pool(name="singles", bufs=1))
        btp = ctx.enter_context(tc.tile_pool(name="btp", bufs=NG * KC))
        expp = ctx.enter_context(tc.tile_pool(name="expp", bufs=4))
        cap = ctx.enter_context(tc.tile_pool(name="cap", bufs=2))
        scr = ctx.enter_context(tc.tile_pool(name="scr", bufs=2))
        psum = ctx.enter_context(tc.tile_pool(name="psum", bufs=2, space="PSUM"))

        at_t = singles.tile([128, KC, 2, M_LOC], FP8)
        bt_tiles = [
            [btp.tile([128, 2, GW], FP8, name=f"bt{g}_{kc}", tag="bt") for kc in range(KC)]
            for g in range(NG)
        ]
        # Load order tuned for fastest first-matmul: the first MM needs only
        # at[mt0, kc0] (32KB) and bt00's first 512 cols (128KB); stream the
        # rest in consumption order.
        nc.sync.dma_start(at_t[:, 0, :, 0:128], at_d[:, 0, :, 0:128])
        nc.sync.dma_start(bt_tiles[0][0][:, :, 0:512], bt_d[0, 0][:, :, 0:512])
        nc.sync.dma_start(at_t[:, 0, :, 128:M_LOC], at_d[:, 0, :, 128:M_LOC])
        nc.sync.dma_start(bt_tiles[0][0][:, :, 512:1024], bt_d[0, 0][:, :, 512:1024])
        nc.sync.dma_start(bt_tiles[0][0][:, :, 1024:GW], bt_d[0, 0][:, :, 1024:GW])
        nc.sync.dma_start(at_t[:, 1], at_d[:, 1])
        for h in range(2):
            nc.sync.dma_start(
                bt_tiles[0][1][:, :, ts(h, GW // 2)], bt_d[0, 1][:, :, ts(h, GW // 2)]
            )
        eye_t = singles.tile([128, 128], F32)
        nc.sync.dma_start(eye_t, eye_d)
        bias_t = singles.tile([128, 1], F32)
        nc.vector.memset(bias_t, -shift)
        rowr = singles.tile([128, MT, NG], F32)
        rowr_l = singles.tile([128, 2], F32)
        diag_sb = singles.tile([128, MT], F32)

        for g in range(1, NG):
            for kc in range(KC):
                nc.sync.dma_start(bt_tiles[g][kc], bt_d[g, kc])

        for g in range(NG):
            colacc = cap.tile([128, GW], BF16, name=f"cacc{g}", tag="cacc")
            for mt in range(MT):
                s_ps = psum.tile([128, GW], F32, name=f"s{g}_{mt}", tag="spsum")
                for kc in range(KC):
                    lhsT = at_t[:, kc, :, ts(mt, 128)]          # [128, 2, 128]
                    for w in range(n_mm):
                        nc.tensor.matmul(
                            s_ps[:, ts(w, mm_w)],
                            lhsT,
                            bt_tiles[g][kc][:, :, ts(w, mm_w)],  # [128, 2, mm_w]
                            start=(kc == 0),
                            stop=(kc == KC - 1),
                            perf_mode=mybir.MatmulPerfMode.DoubleRow,
                        )
                if g == 0:
                    # diag block for mt sits at local cols [mt*128, mt*128+128)
                    dscr = scr.tile([128, 128], F32, name=f"dscr{mt}", tag="dscr")
                    nc.vector.tensor_mul(dscr, s_ps[:, ts(mt, 128)], eye_t)
                    nc.vector.tensor_reduce(
                        out=diag_sb[:, mt : mt + 1],
                        in_=dscr,
                        axis=mybir.AxisListType.X,
                        op=mybir.AluOpType.add,
                    )
                last_slot = g == NG - 1 and mt == MT - 1
                if not last_slot:
                    e_t = expp.tile([128, GW], BF16, name=f"e{g}_{mt}", tag="exp")
                    nc.scalar.activation(
                        e_t,
                        s_ps,
                        mybir.ActivationFunctionType.Exp,
                        bias=bias_t,
                        scale=act_scale,
                        accum_out=rowr[:, mt, g : g + 1],
                    )
                    if mt == 0:
                        nc.vector.tensor_copy(colacc, e_t)
                    else:
                        nc.vector.tensor_add(colacc, colacc, e_t)
                else:
                    # Last slot: process in two half-width pieces so the
                    # first half of the final colacc DMA overlaps the
                    # second half's exp/add (shortens the serial tail).
                    H = GW // 2
                    for h in range(2):
                        e_t = expp.tile([128, H], BF16, name=f"eL{h}", tag="expL")
                        nc.scalar.activation(
                            e_t,
                            s_ps[:, ts(h, H)],
                            mybir.ActivationFunctionType.Exp,
                            bias=bias_t,
                            scale=act_scale,
                            accum_out=rowr_l[:, h : h + 1],
                        )
                        nc.vector.tensor_add(
                            colacc[:, ts(h, H)], colacc[:, ts(h, H)], e_t
                        )
                        nc.sync.dma_start(
                            colsum_d[g][:, ts(h, H)], colacc[:, ts(h, H)]
                        )
            if g != NG - 1:
                nc.sync.dma_start(colsum_d[g], colacc)

        nc.sync.dma_start(rowr_d, rowr)
        nc.sync.dma_start(rowl_d, rowr_l)
        nc.sync.dma_start(diag_d, diag_sb)

    nc.compile()
    return nc


def _prep_inputs(img, txt, scale):
    fp8 = ml_dtypes.float8_e4m3fn
    eye = ((scale / (PRE * PRE)) * np.eye(128)).astype(np.float32)
    in_maps = []
    for c in range(NC):
        A = (PRE * img[c * M_LOC : (c + 1) * M_LOC]).astype(fp8)   # [1024, 512]
        # k = kc*256 + ko*128 + p
        at = np.ascontiguousarray(
            A.T.reshape(KC, 2, 128, M_LOC).transpose(2, 0, 1, 3)
        )                                                          # [128, KC, 2, 1024]
        tr = np.roll(txt, -c * M_LOC, axis=0)                      # local col j -> global (j + c*1024) % N
        B = (PRE * tr).astype(fp8)                                 # [8192, 512]
        bt = np.ascontiguousarray(
            B.T.reshape(KC, 2, 128, NG, GW).transpose(3, 0, 2, 1, 4)
        )                                                          # [NG, KC, 128, 2, GW]
        in_maps.append({"at_in": at, "bt_in": bt, "eye_in": eye})
    return in_maps


def kernel(image_features, text_features, logit_scale):
    global LAST_RESULTS
    img = np.ascontiguousarray(np.asarray(image_features, dtype=np.float32))
    txt = np.ascontiguousarray(np.asarray(text_features, dtype=np.float32))
    scale = float(np.asarray(logit_scale))
    shift = 0.5 * scale

    key = (scale, MM_W)
    if key not in _CACHE:
        _CACHE[key] = _build(scale, shift, MM_W)
    nc = _CACHE[key]

    in_maps = _prep_inputs(img, txt, scale)
    res = run_bass_kernel_spmd(nc, in_maps, core_ids=list(range(NC)))
    LAST_RESULTS = res

    colsum_tot = np.zeros(N, dtype=np.float64)
    lse_rows = []
    diags = []
    for c, r in enumerate(res.results):
        rowr = r["rowr_out"].astype(np.float64)                     # [128, MT, NG]
        rowr[:, MT - 1, NG - 1] = r["rowl_out"].astype(np.float64).sum(axis=1)
        rowsum = rowr.sum(axis=2)                                   # [128, MT]
        lse_rows.append(shift + np.log(rowsum.T.reshape(-1)))       # row = mt*128 + p
        diags.append(r["diag_out"].astype(np.float64).T.reshape(-1))
        colsum_tot += np.roll(
            r["colsum_out"].astype(np.float64).sum(axis=1).reshape(-1), c * M_LOC
        )
    lse_row = np.concatenate(lse_rows)
    diag = np.concatenate(diags)
    lse_col = shift + np.log(colsum_tot)

    loss = 0.5 * (np.mean(lse_row - diag) + np.mean(lse_col - diag))
    return np.float32(loss)
